# revision 3
# baseline (speedup 1.0000x reference)
"""Bidirectional attention kernel for Trainium2 (8 NeuronCores, batch-parallel).

Math (per batch element, all on one core):
    k1p = k1 @ W1 + b1            [N, A]
    k2p = k2 @ W2 + b2            [N, A]
    S   = k1p @ k2p.T             [N, N]
    E   = exp(S)                  (no max-subtraction needed: |S| < ~26)
    o1[m, d] = sum_n E[n, m] v1[n, d] / sum_n E[n, m]   (softmax over N1)
    o2[n, d] = sum_m E[n, m] v2[m, d] / sum_m E[n, m]   (softmax over N2)

Structure (software-pipelined for the For_i steady state):
  A: v loads (v2 first — needed early by the streamed o2 groups)
  B: per score row-block i: score matmuls -> exp on ACT -> PE transposes of
     row-block i-1 (one tile behind exp so the PE never waits on ACT) ->
     streamed o2 output tile i-LAG.  The E^T tiles are per-column-slice, so
     each o2 group's weights come from exactly one transpose burst; the
     whole chain stays in PE program order (no cross-engine head-of-line
     blocking), and the transposes keep the PE warm through the exp window.
  C: o1 output tiles (need all of E) interleaved with NEXT iteration's
     k loads + PE transposes + projections into the other kpT buffer pair
     (loop unrolled 2x so the pairs ping-pong; a prologue seeds pair A).
  Outputs stored bf16 (<0.4% added error vs 2e-2 budget), upcast on host.
  Softmax denominators fold in as a ones-column of v1e/v2e; normalize =
  DVE reciprocal + DVE per-partition scalar multiply.
"""

import numpy as np

import concourse.bass as bass
import concourse.tile as tile
from concourse import bacc, mybir, bass_utils
from concourse.masks import make_identity

N_CORES = 8
B = 8
N = 2048  # N1 == N2
KD = 256  # K1D == K2D
VD = 256  # V1D == V2D
AD = 128
P = 128
LAG = 4  # score row-blocks emitted before the first streamed o2 group

F32 = mybir.dt.float32
F32R = mybir.dt.float32r
BF16 = mybir.dt.bfloat16
AF = mybir.ActivationFunctionType


def _emit_o_group(nc, po_pool, rc_pool, osb_pool, Esrc, ve, o_d, mt, nt,
                  out_row=None):
    """One output tile: 16-deep PSUM accumulation + folded-softmax normalize
    (divide by the ones-column) on DVE, bf16 store on the SP queue.
    `mt` slices Esrc's free dim; `out_row` (default mt) picks the output
    row-block (differs when Esrc is a per-slice ET tile)."""
    if out_row is None:
        out_row = mt
    pot = po_pool.tile([P, VD + 1], F32, tag="po", name="pot")
    for j in range(nt):
        nc.tensor.matmul(
            pot,
            lhsT=Esrc[:, j, 128 * mt : 128 * (mt + 1)],
            rhs=ve[:, j, 0 : VD + 1],
            start=(j == 0),
            stop=(j == nt - 1),
        )
    rc = rc_pool.tile([P, 1], F32, tag="rc", name="rct")
    nc.vector.reciprocal(rc, pot[:, VD : VD + 1])
    ob = osb_pool.tile([P, VD], BF16, tag="ob", name="ob")
    nc.vector.tensor_scalar_mul(ob, pot[:, 0:VD], rc)
    nc.scalar.dma_start(out=o_d[128 * out_row : 128 * (out_row + 1), :], in_=ob)


def _emit_o1_group(nc, po_pool, rc_pool, osb_pool, Es, ve, o_d, mt, nt):
    """o1 output tile mt: lhsT comes from the 16 per-row-block E tiles."""
    pot = po_pool.tile([P, VD + 1], F32, tag="po", name="pot")
    for j in range(nt):
        nc.tensor.matmul(
            pot,
            lhsT=Es[j][:, 128 * mt : 128 * (mt + 1)],
            rhs=ve[:, j, 0 : VD + 1],
            start=(j == 0),
            stop=(j == nt - 1),
        )
    rc = rc_pool.tile([P, 1], F32, tag="rc", name="rct")
    nc.vector.reciprocal(rc, pot[:, VD : VD + 1])
    ob = osb_pool.tile([P, VD], BF16, tag="ob", name="ob")
    nc.vector.tensor_scalar_mul(ob, pot[:, 0:VD], rc)
    nc.scalar.dma_start(out=o_d[128 * mt : 128 * (mt + 1), :], in_=ob)


def _emit_proj_chunk(nc, pools, consts, dram, n, c, kpTs, bias_eng="act"):
    """Load 512 rows of k1+k2, PE-transpose, project into the given
    k1pT/k2pT tiles."""
    stage, ktbuf_pool, ptpp, pscore, po_pool, osb_pool, rc_pool, persist_t = pools
    identity, W1_sb, b1_sb, W2_sb, b2_sb = consts[:5]
    k1pT, k2pT = kpTs
    for k_d, W_sb, b_sb, kpT in (
        (dram["k1"], W1_sb, b1_sb, k1pT),
        (dram["k2"], W2_sb, b2_sb, k2pT),
    ):
        st = stage.tile([P, 4, KD], F32, tag="stage", name="st")
        nc.scalar.dma_start(
            out=st,
            in_=k_d[512 * c : 512 * (c + 1), :].rearrange("(t p) k -> p t k", p=P),
        )
        kt = ktbuf_pool.tile([P, 2, 512], F32R, tag="kt", name="kt")
        for kb in range(2):
            pt = ptpp.tile([P, 512], F32, tag="pt512", name="pt")
            for t in range(4):
                nc.tensor.transpose(
                    pt[:, 128 * t : 128 * (t + 1)],
                    st[:, t, 128 * kb : 128 * (kb + 1)],
                    identity,
                )
            nc.vector.tensor_copy(kt[:, kb, :], pt)
        pp = ptpp.tile([P, 512], F32, tag="pt512", name="pp")
        for kb in range(2):
            nc.tensor.matmul(
                pp, lhsT=W_sb[:, kb, :], rhs=kt[:, kb, :],
                start=(kb == 0), stop=(kb == 1),
            )
        if bias_eng == "act":
            nc.scalar.activation(
                kpT[:, 512 * c : 512 * (c + 1)], pp, AF.Identity, bias=b_sb,
                scale=1.0,
            )
        else:
            nc.vector.tensor_scalar_add(kpT[:, 512 * c : 512 * (c + 1)], pp, b_sb)


def _emit_body_pools(nc, tc, consts, persist_t, pools, dram, n, kpT_cur,
                     kpT_next, proj_tail=True, stream_o2=True,
                     xbar_queue="sync", proj_in_b=False, o2_burst=0,
                     skip_v=False, etrans="pe"):
    """One software-pipelined body pass. Reads kpT_cur (seeded by prologue
    or the previous body), writes the NEXT iteration's projections into
    kpT_next (no WAR against this body's score reads — separate buffers).
    proj_in_b: interleave next iteration's projections into phase B with
    bias on DVE (keeps ACT = exp only); else they trail phase C.
    o2_burst: if >0, emit o2 groups in bursts of this size inside B."""
    nt = n // P
    nch = n // 512
    stage, ktbuf_pool, ptpp, pscore, po_pool, osb_pool, rc_pool, _ = pools
    k1pT, k2pT = kpT_cur
    Es, ETs = persist_t["Es"], persist_t["ETs"]
    v1e, v2e = persist_t["v1e"], persist_t["v2e"]
    o1_d, o2_d = dram["o1"], dram["o2"]

    # ---- A: v loads; v2 first (streamed o2 groups read it early) ----
    for v_d, ve in (() if skip_v else ((dram["v2"], v2e), (dram["v1"], v1e))):
        nc.vector.memset(ve[:, :, VD : VD + 2], 1.0)
        for c in range(nch):
            sv = stage.tile([P, 4, VD], F32, tag="stage", name="sv")
            nc.scalar.dma_start(
                out=sv,
                in_=v_d[512 * c : 512 * (c + 1), :].rearrange("(t p) d -> p t d", p=P),
            )
            nc.any.tensor_copy(ve[:, 4 * c : 4 * (c + 1), 0:VD], sv)

    # ---- B: score + exp + xbar-transpose + streamed o2 groups ----
    def emit_transposes(i):
        if i >= 0:
            # PE-mode transposes fill the exp-bound window and keep HAM warm;
            # psum slots shared with the score pool (tag rotation)
            id_bf = consts[5]
            for jb in range(4):
                pt = pscore.tile([P, 512], BF16, tag="ps", name="ptet")
                for t in range(4):
                    j = 4 * jb + t
                    nc.tensor.transpose(
                        pt[:, 128 * t : 128 * (t + 1)],
                        Es[i][:, 128 * j : 128 * (j + 1)],
                        id_bf,
                    )
                nc.vector.tensor_copy(
                    ETs[i][:, 4 * jb : 4 * (jb + 1), :], pt)

    def emit_score_tile(i, transpose_of=None):
        for h in range(2):
            ps = pscore.tile([P, 1024], F32, tag="ps", name="ps")
            for q in range(2):
                col = 1024 * h + 512 * q
                nc.tensor.matmul(
                    ps[:, 512 * q : 512 * (q + 1)],
                    lhsT=k1pT[:, 128 * i : 128 * (i + 1)],
                    rhs=k2pT[:, col : col + 512],
                    start=True,
                    stop=True,
                )
            nc.scalar.activation(Es[i][:, 1024 * h : 1024 * (h + 1)], ps, AF.Exp)
        if etrans == "xbar":
            qeng = nc.scalar if xbar_queue == "scalar" else nc.sync
            qeng.dma_start_transpose(ETs[i], Es[i])
        else:
            emit_transposes(transpose_of if transpose_of is not None else i)

    def o2_group(g):
        # o2 tile g's lhsT set is exactly ETs[g] (written by one transpose)
        _emit_o_group(nc, po_pool, rc_pool, osb_pool, ETs[g], v2e, o2_d, 0, nt,
                      out_row=g)

    def proj_chunk(c):
        _emit_proj_chunk(nc, pools, consts, dram, n, c, kpT_next,
                         bias_eng="dve" if proj_in_b else "act")

    next_g = 0  # next o2 group to emit
    for i in range(nt):
        # PE transposes run one tile behind exp so the PE never waits on ACT
        emit_score_tile(i, transpose_of=(i - 1 if etrans == "pe" else i))
        if proj_in_b and proj_tail and i % 4 == 1:
            proj_chunk(i // 4)
        if stream_o2:
            burst = o2_burst if o2_burst > 0 else 1
            avail = i - LAG + 1 - (1 if etrans == "pe" else 0)
            while next_g + burst <= avail:
                for g in range(next_g, next_g + burst):
                    o2_group(g)
                next_g += burst
    if etrans == "pe":
        emit_transposes(nt - 1)
    for g in range(next_g, nt):
        o2_group(g)

    # ---- C: o1 groups (+ next projections when not interleaved into B) ----
    for mt in range(nt):
        _emit_o1_group(nc, po_pool, rc_pool, osb_pool, Es, v1e, o1_d, mt, nt)
        if not proj_in_b and proj_tail and mt % 4 == 3:
            proj_chunk(mt // 4)


def _build(n, iters=None, reps=1, stream_o2=True, xbar_queue="sync",
           proj_in_b=False, o2_burst=0, skip_v=False, loop_proj=True,
           etrans="pe"):
    """iters=None: single-pass body (reps repeats). iters=k: For_i loop."""
    from contextlib import ExitStack

    nc = bacc.Bacc("TRN2", target_bir_lowering=False, debug=False)
    dram = {
        "k1": nc.dram_tensor("k1", [n, KD], F32, kind="ExternalInput").ap(),
        "k2": nc.dram_tensor("k2", [n, KD], F32, kind="ExternalInput").ap(),
        "v1": nc.dram_tensor("v1", [n, VD], F32, kind="ExternalInput").ap(),
        "v2": nc.dram_tensor("v2", [n, VD], F32, kind="ExternalInput").ap(),
        "o1": nc.dram_tensor("o1", [n, VD], BF16, kind="ExternalOutput").ap(),
        "o2": nc.dram_tensor("o2", [n, VD], BF16, kind="ExternalOutput").ap(),
    }
    W1_d = nc.dram_tensor("W1", [KD, AD], F32R, kind="ExternalInput").ap()
    b1_d = nc.dram_tensor("b1", [AD], F32, kind="ExternalInput").ap()
    W2_d = nc.dram_tensor("W2", [KD, AD], F32R, kind="ExternalInput").ap()
    b2_d = nc.dram_tensor("b2", [AD], F32, kind="ExternalInput").ap()

    nt = n // P
    with tile.TileContext(nc) as tc:
        with ExitStack() as es:
            consts_pool = es.enter_context(tc.tile_pool(name="consts", bufs=1))
            persist = es.enter_context(tc.tile_pool(name="persist", bufs=1))
            identity = consts_pool.tile([P, P], F32)
            make_identity(nc, identity)
            id_bf = consts_pool.tile([P, P], BF16)
            make_identity(nc, id_bf)
            # consts via SWDGE (gpsimd) so they don't queue ahead of k loads
            W1_sb = consts_pool.tile([P, 2, AD], F32R)
            nc.gpsimd.dma_start(out=W1_sb, in_=W1_d.rearrange("(kb k) a -> k kb a", k=P))
            W2_sb = consts_pool.tile([P, 2, AD], F32R)
            nc.gpsimd.dma_start(out=W2_sb, in_=W2_d.rearrange("(kb k) a -> k kb a", k=P))
            b1_sb = consts_pool.tile([P, 1], F32)
            nc.gpsimd.dma_start(out=b1_sb, in_=b1_d.rearrange("(a one) -> a one", one=1))
            b2_sb = consts_pool.tile([P, 1], F32)
            nc.gpsimd.dma_start(out=b2_sb, in_=b2_d.rearrange("(a one) -> a one", one=1))
            consts = (identity, W1_sb, b1_sb, W2_sb, b2_sb, id_bf)

            kpT_a = (
                persist.tile([P, n], F32R, tag="k1pTa", name="k1pTa"),
                persist.tile([P, n], F32R, tag="k2pTa", name="k2pTa"),
            )
            kpT_b = (
                persist.tile([P, n], F32R, tag="k1pTb", name="k1pTb"),
                persist.tile([P, n], F32R, tag="k2pTb", name="k2pTb"),
            )
            persist_t = {
                # E as one tile per 128-row block: exact deps for the o1
                # groups and the next iteration's exp overwrites
                "Es": [
                    persist.tile([P, n], BF16, tag=f"E{i}", name=f"E{i}")
                    for i in range(nt)
                ],
                # E^T as one tile per 128-column slice so each xbar transpose
                # and each streamed o2 group get exact (not whole-tile) deps
                "ETs": [
                    persist.tile([P, nt, P], BF16, tag=f"ET{i}", name=f"ET{i}")
                    for i in range(nt)
                ],
                "v1e": persist.tile([P, nt, VD + 2], BF16, tag="v1e", name="v1e"),
                "v2e": persist.tile([P, nt, VD + 2], BF16, tag="v2e", name="v2e"),
            }

            stage = es.enter_context(tc.tile_pool(name="stage", bufs=3))
            ktbuf_pool = es.enter_context(tc.tile_pool(name="ktbuf", bufs=2))
            ptpp = es.enter_context(tc.tile_pool(name="ptpp", bufs=2, space="PSUM"))
            pscore = es.enter_context(tc.tile_pool(name="pscore", bufs=2, space="PSUM"))
            po_pool = es.enter_context(tc.tile_pool(name="po", bufs=2, space="PSUM"))
            osb_pool = es.enter_context(tc.tile_pool(name="osb", bufs=4))
            rc_pool = es.enter_context(tc.tile_pool(name="rc", bufs=4))
            pools = (stage, ktbuf_pool, ptpp, pscore, po_pool, osb_pool, rc_pool,
                     persist_t)

            # prologue: seed buffer A for the first body
            for c in range(n // 512):
                _emit_proj_chunk(nc, pools, consts, dram, n, c, kpT_a)

            kw = dict(stream_o2=stream_o2, xbar_queue=xbar_queue,
                      proj_in_b=proj_in_b, o2_burst=o2_burst, skip_v=skip_v,
                      etrans=etrans)
            if iters is None:
                for r in range(reps):
                    cur, nxt = (kpT_a, kpT_b) if r % 2 == 0 else (kpT_b, kpT_a)
                    _emit_body_pools(nc, tc, consts, persist_t, pools, dram, n,
                                     cur, nxt, proj_tail=(r < reps - 1), **kw)
            else:
                # unrolled by 2 so the A/B kpT buffers ping-pong across the
                # For_i iterations (addresses are baked per emission)
                assert iters % 2 == 0
                with tc.For_i(0, iters // 2, 1):
                    _emit_body_pools(nc, tc, consts, persist_t, pools, dram, n,
                                     kpT_a, kpT_b, proj_tail=loop_proj, **kw)
                    _emit_body_pools(nc, tc, consts, persist_t, pools, dram, n,
                                     kpT_b if loop_proj else kpT_a, kpT_a,
                                     proj_tail=loop_proj, **kw)

    nc.compile()
    return nc


def build_nc(n: int = N, reps: int = 1, **kw):
    return _build(n, iters=None, reps=reps, **kw)


def build_nc_loop(n: int = N, iters: int = 16, **kw):
    return _build(n, iters=iters, **kw)


_NC_CACHE: dict = {}


def _get_nc(n: int = N):
    if n not in _NC_CACHE:
        _NC_CACHE[n] = build_nc(n)
    return _NC_CACHE[n]


def kernel(k1, k2, v1, v2, W1, b1, W2, b2):
    """Full-input entry point: shard batch across 8 cores, run SPMD, gather."""
    nc = _get_nc(N)
    k1 = np.ascontiguousarray(np.asarray(k1, dtype=np.float32))
    k2 = np.ascontiguousarray(np.asarray(k2, dtype=np.float32))
    v1 = np.ascontiguousarray(np.asarray(v1, dtype=np.float32))
    v2 = np.ascontiguousarray(np.asarray(v2, dtype=np.float32))
    W1 = np.ascontiguousarray(np.asarray(W1, dtype=np.float32))
    b1 = np.ascontiguousarray(np.asarray(b1, dtype=np.float32))
    W2 = np.ascontiguousarray(np.asarray(W2, dtype=np.float32))
    b2 = np.ascontiguousarray(np.asarray(b2, dtype=np.float32))
    in_maps = [
        {
            "k1": k1[c], "k2": k2[c], "v1": v1[c], "v2": v2[c],
            "W1": W1, "b1": b1, "W2": W2, "b2": b2,
        }
        for c in range(N_CORES)
    ]
    res = bass_utils.run_bass_kernel_spmd(nc, in_maps, core_ids=list(range(N_CORES)))
    o2 = np.stack([res.results[c]["o2"].astype(np.float32) for c in range(N_CORES)])
    o1 = np.stack([res.results[c]["o1"].astype(np.float32) for c in range(N_CORES)])
    return (o2, o1)


# revision 4
# speedup vs baseline: 1.0981x; 1.0981x over previous
"""Bidirectional attention kernel for Trainium2 (8 NeuronCores, batch-parallel).

Math (per batch element, all on one core):
    k1p = k1 @ W1 + b1            [N, A]
    k2p = k2 @ W2 + b2            [N, A]
    S   = k1p @ k2p.T             [N, N]
    E   = exp(S)                  (no max-subtraction needed: |S| < ~26)
    o1[m, d] = sum_n E[n, m] v1[n, d] / sum_n E[n, m]   (softmax over N1)
    o2[n, d] = sum_m E[n, m] v2[m, d] / sum_m E[n, m]   (softmax over N2)

Structure (software-pipelined for the For_i steady state):
  A: v loads (v2 first — needed early by the streamed o2 groups)
  B: per score row-block i: score matmuls -> exp on ACT -> PE transposes of
     row-block i-1 (one tile behind exp so the PE never waits on ACT) ->
     streamed o2 output tile i-LAG.  The E^T tiles are per-column-slice, so
     each o2 group's weights come from exactly one transpose burst; the
     whole chain stays in PE program order (no cross-engine head-of-line
     blocking), and the transposes keep the PE warm through the exp window.
  C: o1 output tiles (need all of E) interleaved with NEXT iteration's
     k loads + PE transposes + projections into the other kpT buffer pair
     (loop unrolled 2x so the pairs ping-pong; a prologue seeds pair A).
  Outputs stored bf16 (<0.4% added error vs 2e-2 budget), upcast on host.
  Softmax denominators fold in as a ones-column of v1e/v2e; normalize =
  DVE reciprocal + DVE per-partition scalar multiply.
"""

import numpy as np

import concourse.bass as bass
import concourse.tile as tile
from concourse import bacc, mybir, bass_utils
from concourse.masks import make_identity

N_CORES = 8
B = 8
N = 2048  # N1 == N2
KD = 256  # K1D == K2D
VD = 256  # V1D == V2D
AD = 128
P = 128
LAG = 6  # score row-blocks emitted before the first streamed o2 group

F32 = mybir.dt.float32
F32R = mybir.dt.float32r
BF16 = mybir.dt.bfloat16
AF = mybir.ActivationFunctionType


def _emit_o_group(nc, po_pool, rc_pool, osb_pool, Esrc, ve, o_d, mt, nt,
                  out_row=None):
    """One output tile: 16-deep PSUM accumulation + folded-softmax normalize
    (divide by the ones-column) on DVE, bf16 store on the SP queue.
    `mt` slices Esrc's free dim; `out_row` (default mt) picks the output
    row-block (differs when Esrc is a per-slice ET tile)."""
    if out_row is None:
        out_row = mt
    pot = po_pool.tile([P, VD + 1], F32, tag="po", name="pot")
    for j in range(nt):
        nc.tensor.matmul(
            pot,
            lhsT=Esrc[:, j, 128 * mt : 128 * (mt + 1)],
            rhs=ve[:, j, 0 : VD + 1],
            start=(j == 0),
            stop=(j == nt - 1),
        )
    rc = rc_pool.tile([P, 1], F32, tag="rc", name="rct")
    nc.vector.reciprocal(rc, pot[:, VD : VD + 1])
    ob = osb_pool.tile([P, VD], BF16, tag="ob", name="ob")
    nc.vector.tensor_scalar_mul(ob, pot[:, 0:VD], rc)
    nc.scalar.dma_start(out=o_d[128 * out_row : 128 * (out_row + 1), :], in_=ob)


def _emit_o1_group(nc, po_pool, rc_pool, osb_pool, Es, ve, o_d, mt, nt):
    """o1 output tile mt: lhsT comes from the 16 per-row-block E tiles."""
    pot = po_pool.tile([P, VD + 1], F32, tag="po", name="pot")
    for j in range(nt):
        nc.tensor.matmul(
            pot,
            lhsT=Es[j][:, 128 * mt : 128 * (mt + 1)],
            rhs=ve[:, j, 0 : VD + 1],
            start=(j == 0),
            stop=(j == nt - 1),
        )
    rc = rc_pool.tile([P, 1], F32, tag="rc", name="rct")
    nc.vector.reciprocal(rc, pot[:, VD : VD + 1])
    ob = osb_pool.tile([P, VD], BF16, tag="ob", name="ob")
    nc.vector.tensor_scalar_mul(ob, pot[:, 0:VD], rc)
    nc.scalar.dma_start(out=o_d[128 * mt : 128 * (mt + 1), :], in_=ob)


def _emit_proj_chunk(nc, pools, consts, dram, n, c, kpTs, bias_eng="act"):
    """Load 512 rows of k1+k2, PE-transpose, project into the given
    k1pT/k2pT tiles."""
    stage, ktbuf_pool, ptpp, pscore, po_pool, osb_pool, rc_pool, persist_t = pools
    identity, W1_sb, b1_sb, W2_sb, b2_sb = consts[:5]
    k1pT, k2pT = kpTs
    for k_d, W_sb, b_sb, kpT in (
        (dram["k1"], W1_sb, b1_sb, k1pT),
        (dram["k2"], W2_sb, b2_sb, k2pT),
    ):
        st = stage.tile([P, 4, KD], F32, tag="stage", name="st")
        nc.scalar.dma_start(
            out=st,
            in_=k_d[512 * c : 512 * (c + 1), :].rearrange("(t p) k -> p t k", p=P),
        )
        kt = ktbuf_pool.tile([P, 2, 512], F32R, tag="kt", name="kt")
        for kb in range(2):
            pt = ptpp.tile([P, 512], F32, tag="pt512", name="pt")
            for t in range(4):
                nc.tensor.transpose(
                    pt[:, 128 * t : 128 * (t + 1)],
                    st[:, t, 128 * kb : 128 * (kb + 1)],
                    identity,
                )
            nc.vector.tensor_copy(kt[:, kb, :], pt)
        pp = ptpp.tile([P, 512], F32, tag="pt512", name="pp")
        for kb in range(2):
            nc.tensor.matmul(
                pp, lhsT=W_sb[:, kb, :], rhs=kt[:, kb, :],
                start=(kb == 0), stop=(kb == 1),
            )
        if bias_eng == "act":
            nc.scalar.activation(
                kpT[:, 512 * c : 512 * (c + 1)], pp, AF.Identity, bias=b_sb,
                scale=1.0,
            )
        else:
            nc.vector.tensor_scalar_add(kpT[:, 512 * c : 512 * (c + 1)], pp, b_sb)


def _emit_body_pools(nc, tc, consts, persist_t, pools, dram, n, kpT_cur,
                     kpT_next, proj_tail=True, stream_o2=True,
                     xbar_queue="sync", proj_in_b=False, o2_burst=0,
                     skip_v=False, etrans="pe"):
    """One software-pipelined body pass. Reads kpT_cur (seeded by prologue
    or the previous body), writes the NEXT iteration's projections into
    kpT_next (no WAR against this body's score reads — separate buffers).
    proj_in_b: interleave next iteration's projections into phase B with
    bias on DVE (keeps ACT = exp only); else they trail phase C.
    o2_burst: if >0, emit o2 groups in bursts of this size inside B."""
    nt = n // P
    nch = n // 512
    stage, ktbuf_pool, ptpp, pscore, po_pool, osb_pool, rc_pool, _ = pools
    k1pT, k2pT = kpT_cur
    Es, ETs = persist_t["Es"], persist_t["ETs"]
    v1e, v2e = persist_t["v1e"], persist_t["v2e"]
    o1_d, o2_d = dram["o1"], dram["o2"]

    # ---- A: v loads; v2 first (streamed o2 groups read it early) ----
    for v_d, ve in (() if skip_v else ((dram["v2"], v2e), (dram["v1"], v1e))):
        nc.vector.memset(ve[:, :, VD : VD + 2], 1.0)
        for c in range(nch):
            sv = stage.tile([P, 4, VD], F32, tag="stage", name="sv")
            nc.scalar.dma_start(
                out=sv,
                in_=v_d[512 * c : 512 * (c + 1), :].rearrange("(t p) d -> p t d", p=P),
            )
            nc.any.tensor_copy(ve[:, 4 * c : 4 * (c + 1), 0:VD], sv)

    # ---- B: score + exp + xbar-transpose + streamed o2 groups ----
    def emit_transposes(i):
        if i >= 0:
            # PE-mode transposes fill the exp-bound window and keep HAM warm;
            # psum slots shared with the score pool (tag rotation)
            id_bf = consts[5]
            for jb in range(4):
                pt = pscore.tile([P, 512], BF16, tag="ps", name="ptet")
                for t in range(4):
                    j = 4 * jb + t
                    nc.tensor.transpose(
                        pt[:, 128 * t : 128 * (t + 1)],
                        Es[i][:, 128 * j : 128 * (j + 1)],
                        id_bf,
                    )
                nc.vector.tensor_copy(
                    ETs[i][:, 4 * jb : 4 * (jb + 1), :], pt)

    def emit_score_tile(i, transpose_of=None):
        for h in range(2):
            ps = pscore.tile([P, 1024], F32, tag="ps", name="ps")
            for q in range(2):
                col = 1024 * h + 512 * q
                nc.tensor.matmul(
                    ps[:, 512 * q : 512 * (q + 1)],
                    lhsT=k1pT[:, 128 * i : 128 * (i + 1)],
                    rhs=k2pT[:, col : col + 512],
                    start=True,
                    stop=True,
                )
            nc.scalar.activation(Es[i][:, 1024 * h : 1024 * (h + 1)], ps, AF.Exp)
        if etrans == "xbar":
            qeng = nc.scalar if xbar_queue == "scalar" else nc.sync
            qeng.dma_start_transpose(ETs[i], Es[i])
        else:
            emit_transposes(transpose_of if transpose_of is not None else i)

    def o2_group(g):
        # o2 tile g's lhsT set is exactly ETs[g] (written by one transpose)
        _emit_o_group(nc, po_pool, rc_pool, osb_pool, ETs[g], v2e, o2_d, 0, nt,
                      out_row=g)

    def proj_chunk(c):
        _emit_proj_chunk(nc, pools, consts, dram, n, c, kpT_next,
                         bias_eng="dve" if proj_in_b else "act")

    next_g = 0  # next o2 group to emit
    for i in range(nt):
        # PE transposes run one tile behind exp so the PE never waits on ACT
        emit_score_tile(i, transpose_of=(i - 1 if etrans == "pe" else i))
        if proj_in_b and proj_tail and i % 4 == 1:
            proj_chunk(i // 4)
        if stream_o2:
            burst = o2_burst if o2_burst > 0 else 1
            avail = i - LAG + 1 - (1 if etrans == "pe" else 0)
            while next_g + burst <= avail:
                for g in range(next_g, next_g + burst):
                    o2_group(g)
                next_g += burst
    if etrans == "pe":
        emit_transposes(nt - 1)
    for g in range(next_g, nt):
        o2_group(g)

    # ---- C: o1 groups (+ next projections when not interleaved into B) ----
    for mt in range(nt):
        _emit_o1_group(nc, po_pool, rc_pool, osb_pool, Es, v1e, o1_d, mt, nt)
        if not proj_in_b and proj_tail and mt % 4 == 3:
            proj_chunk(mt // 4)


def _build(n, iters=None, reps=1, stream_o2=True, xbar_queue="sync",
           proj_in_b=False, o2_burst=0, skip_v=False, loop_proj=True,
           etrans="pe"):
    """iters=None: single-pass body (reps repeats). iters=k: For_i loop."""
    from contextlib import ExitStack

    nc = bacc.Bacc("TRN2", target_bir_lowering=False, debug=False)
    dram = {
        "k1": nc.dram_tensor("k1", [n, KD], F32, kind="ExternalInput").ap(),
        "k2": nc.dram_tensor("k2", [n, KD], F32, kind="ExternalInput").ap(),
        "v1": nc.dram_tensor("v1", [n, VD], F32, kind="ExternalInput").ap(),
        "v2": nc.dram_tensor("v2", [n, VD], F32, kind="ExternalInput").ap(),
        "o1": nc.dram_tensor("o1", [n, VD], BF16, kind="ExternalOutput").ap(),
        "o2": nc.dram_tensor("o2", [n, VD], BF16, kind="ExternalOutput").ap(),
    }
    W1_d = nc.dram_tensor("W1", [KD, AD], F32R, kind="ExternalInput").ap()
    b1_d = nc.dram_tensor("b1", [AD], F32, kind="ExternalInput").ap()
    W2_d = nc.dram_tensor("W2", [KD, AD], F32R, kind="ExternalInput").ap()
    b2_d = nc.dram_tensor("b2", [AD], F32, kind="ExternalInput").ap()

    nt = n // P
    with tile.TileContext(nc) as tc:
        with ExitStack() as es:
            consts_pool = es.enter_context(tc.tile_pool(name="consts", bufs=1))
            persist = es.enter_context(tc.tile_pool(name="persist", bufs=1))
            identity = consts_pool.tile([P, P], F32)
            make_identity(nc, identity)
            id_bf = consts_pool.tile([P, P], BF16)
            make_identity(nc, id_bf)
            # consts via SWDGE (gpsimd) so they don't queue ahead of k loads
            W1_sb = consts_pool.tile([P, 2, AD], F32R)
            nc.gpsimd.dma_start(out=W1_sb, in_=W1_d.rearrange("(kb k) a -> k kb a", k=P))
            W2_sb = consts_pool.tile([P, 2, AD], F32R)
            nc.gpsimd.dma_start(out=W2_sb, in_=W2_d.rearrange("(kb k) a -> k kb a", k=P))
            b1_sb = consts_pool.tile([P, 1], F32)
            nc.gpsimd.dma_start(out=b1_sb, in_=b1_d.rearrange("(a one) -> a one", one=1))
            b2_sb = consts_pool.tile([P, 1], F32)
            nc.gpsimd.dma_start(out=b2_sb, in_=b2_d.rearrange("(a one) -> a one", one=1))
            consts = (identity, W1_sb, b1_sb, W2_sb, b2_sb, id_bf)

            kpT_a = (
                persist.tile([P, n], F32R, tag="k1pTa", name="k1pTa"),
                persist.tile([P, n], F32R, tag="k2pTa", name="k2pTa"),
            )
            kpT_b = (
                persist.tile([P, n], F32R, tag="k1pTb", name="k1pTb"),
                persist.tile([P, n], F32R, tag="k2pTb", name="k2pTb"),
            )
            persist_t = {
                # E as one tile per 128-row block: exact deps for the o1
                # groups and the next iteration's exp overwrites
                "Es": [
                    persist.tile([P, n], BF16, tag=f"E{i}", name=f"E{i}")
                    for i in range(nt)
                ],
                # E^T as one tile per 128-column slice so each xbar transpose
                # and each streamed o2 group get exact (not whole-tile) deps
                "ETs": [
                    persist.tile([P, nt, P], BF16, tag=f"ET{i}", name=f"ET{i}")
                    for i in range(nt)
                ],
                "v1e": persist.tile([P, nt, VD + 2], BF16, tag="v1e", name="v1e"),
                "v2e": persist.tile([P, nt, VD + 2], BF16, tag="v2e", name="v2e"),
            }

            stage = es.enter_context(tc.tile_pool(name="stage", bufs=3))
            ktbuf_pool = es.enter_context(tc.tile_pool(name="ktbuf", bufs=2))
            ptpp = es.enter_context(tc.tile_pool(name="ptpp", bufs=2, space="PSUM"))
            pscore = es.enter_context(tc.tile_pool(name="pscore", bufs=2, space="PSUM"))
            po_pool = es.enter_context(tc.tile_pool(name="po", bufs=2, space="PSUM"))
            osb_pool = es.enter_context(tc.tile_pool(name="osb", bufs=4))
            rc_pool = es.enter_context(tc.tile_pool(name="rc", bufs=4))
            pools = (stage, ktbuf_pool, ptpp, pscore, po_pool, osb_pool, rc_pool,
                     persist_t)

            # prologue: seed buffer A for the first body
            for c in range(n // 512):
                _emit_proj_chunk(nc, pools, consts, dram, n, c, kpT_a)

            kw = dict(stream_o2=stream_o2, xbar_queue=xbar_queue,
                      proj_in_b=proj_in_b, o2_burst=o2_burst, skip_v=skip_v,
                      etrans=etrans)
            if iters is None:
                for r in range(reps):
                    cur, nxt = (kpT_a, kpT_b) if r % 2 == 0 else (kpT_b, kpT_a)
                    _emit_body_pools(nc, tc, consts, persist_t, pools, dram, n,
                                     cur, nxt, proj_tail=(r < reps - 1), **kw)
            else:
                # unrolled by 2 so the A/B kpT buffers ping-pong across the
                # For_i iterations (addresses are baked per emission)
                assert iters % 2 == 0
                with tc.For_i(0, iters // 2, 1):
                    _emit_body_pools(nc, tc, consts, persist_t, pools, dram, n,
                                     kpT_a, kpT_b, proj_tail=loop_proj, **kw)
                    _emit_body_pools(nc, tc, consts, persist_t, pools, dram, n,
                                     kpT_b if loop_proj else kpT_a, kpT_a,
                                     proj_tail=loop_proj, **kw)

    nc.compile()
    return nc


def build_nc(n: int = N, reps: int = 1, **kw):
    return _build(n, iters=None, reps=reps, **kw)


def build_nc_loop(n: int = N, iters: int = 16, **kw):
    return _build(n, iters=iters, **kw)


_NC_CACHE: dict = {}


def _get_nc(n: int = N):
    if n not in _NC_CACHE:
        _NC_CACHE[n] = build_nc(n)
    return _NC_CACHE[n]


def kernel(k1, k2, v1, v2, W1, b1, W2, b2):
    """Full-input entry point: shard batch across 8 cores, run SPMD, gather."""
    nc = _get_nc(N)
    k1 = np.ascontiguousarray(np.asarray(k1, dtype=np.float32))
    k2 = np.ascontiguousarray(np.asarray(k2, dtype=np.float32))
    v1 = np.ascontiguousarray(np.asarray(v1, dtype=np.float32))
    v2 = np.ascontiguousarray(np.asarray(v2, dtype=np.float32))
    W1 = np.ascontiguousarray(np.asarray(W1, dtype=np.float32))
    b1 = np.ascontiguousarray(np.asarray(b1, dtype=np.float32))
    W2 = np.ascontiguousarray(np.asarray(W2, dtype=np.float32))
    b2 = np.ascontiguousarray(np.asarray(b2, dtype=np.float32))
    in_maps = [
        {
            "k1": k1[c], "k2": k2[c], "v1": v1[c], "v2": v2[c],
            "W1": W1, "b1": b1, "W2": W2, "b2": b2,
        }
        for c in range(N_CORES)
    ]
    res = bass_utils.run_bass_kernel_spmd(nc, in_maps, core_ids=list(range(N_CORES)))
    o2 = np.stack([res.results[c]["o2"].astype(np.float32) for c in range(N_CORES)])
    o1 = np.stack([res.results[c]["o1"].astype(np.float32) for c in range(N_CORES)])
    return (o2, o1)
